# revision 39
# baseline (speedup 1.0000x reference)
"""Trainium2 Bass kernel for nn_Model_17325898072228 (attention-MIL pooling).

Math (per batch b, class c):
    h      = relu(bags[b] @ W1[c] + b1[c])            # [N, I]
    s      = relu(h @ Wa[c] + ba[c])                  # [N]
    w      = softmax(s)                               # [N]
    pooled = w @ h                                    # [I]
    y      = sigmoid(Wp[:I]@pooled + Wp[I:]@loc[b] + bp)

Key algebraic reduction used on-device: the pooled vector is never needed,
only its dot with Wp:
    Wp_I @ pooled = (sum_n e_n * q_n) / (sum_n e_n)
with e_n = exp(relu(s_n)) (softmax shift is unnecessary: scores are O(1))
and q_n = Wp_I @ h_n.  Both s and q are j-contractions of h, so one matmul
per (batch, class) with a [128, 112] block-column stationary accumulates,
for a PAIR of batches, s rows at PSUM partitions {0-15, 32-47} and q rows
at {64-79, 96-111} for all 16 classes.  All partition bases are 32-aligned
(a hardware requirement for wide PSUM reads), junk rows in between are
processed harmlessly in parallel, and the whole softmax/dot epilogue runs
once per batch-pair instead of once per batch.  No transpose of h is ever
required.

Sharding: data-parallel over batch B=32 across 8 cores (4 batches each).
All weights are replicated; inputs are laid out host-side (transposes,
bf16 casts, block-column stationary assembly) so on-device DMA is
contiguous.
"""

import sys

sys.path.insert(0, "/opt/trn_rl_repo")

from contextlib import ExitStack

import ml_dtypes
import numpy as np

import concourse.bass as bass  # noqa: F401  (registers engines)
import concourse.tile as tile
from concourse import bacc, bass_utils, masks, mybir


B, N, I, L, C = 32, 1024, 128, 32, 16
NCORES = 8
BLOC = B // NCORES  # batches per core
NGRP = BLOC // 2  # batch pairs per core
HALF = 512  # matmul moving-operand max free dim (= one fp32 psum bank)
M2 = 112  # stationary width for the paired s/q matmul
NU = 2 * C  # (class, parity) units per batch-pair

F32 = mybir.dt.float32
BF16 = mybir.dt.bfloat16
AF = mybir.ActivationFunctionType
OP = mybir.AluOpType


def _build_kernel(tc):
    nc = tc.nc
    xt_d = nc.dram_tensor("xt", [I, BLOC * N], BF16, kind="ExternalInput").ap()
    w1s_d = nc.dram_tensor("w1s", [I, C * I], BF16, kind="ExternalInput").ap()
    b1t_d = nc.dram_tensor("b1t", [I, C], F32, kind="ExternalInput").ap()
    sqw_d = nc.dram_tensor("sqw", [I, NU * M2], BF16, kind="ExternalInput").ap()
    ba48_d = nc.dram_tensor("ba48", [48, 1], F32, kind="ExternalInput").ap()
    loct_d = nc.dram_tensor("loct", [L, BLOC], F32, kind="ExternalInput").ap()
    wpl_d = nc.dram_tensor("wpl", [L, 1], F32, kind="ExternalInput").ap()
    sel_d = nc.dram_tensor("sel", [BLOC, NGRP * 48], F32, kind="ExternalInput").ap()
    bp_d = nc.dram_tensor("bp", [BLOC, 1], F32, kind="ExternalInput").ap()
    y_d = nc.dram_tensor("y", [BLOC, C], F32, kind="ExternalOutput").ap()

    with ExitStack() as ctx:
        consts = ctx.enter_context(tc.tile_pool(name="consts", bufs=1))
        zpool = ctx.enter_context(tc.tile_pool(name="z", bufs=6, space="PSUM"))
        sqpool = ctx.enter_context(tc.tile_pool(name="sq", bufs=1, space="PSUM"))
        htpool = ctx.enter_context(tc.tile_pool(name="ht", bufs=12))
        epool = ctx.enter_context(tc.tile_pool(name="ep", bufs=3))

        # ---- persistent loads, ordered/split so the first matmuls can
        # start early; spread across both HWDGE rings (sync + scalar) ----
        w1s = consts.tile([I, C * I], BF16)
        nc.sync.dma_start(w1s[:, :I], w1s_d[:, :I])
        xt = consts.tile([I, BLOC * N], BF16)
        nc.scalar.dma_start(xt[:, :HALF], xt_d[:, :HALF])
        nc.sync.dma_start(xt[:, N : N + HALF], xt_d[:, N : N + HALF])
        b1t = consts.tile([I, C], F32)
        nc.scalar.dma_start(b1t[:], b1t_d)
        sqw = consts.tile([I, NU * M2], BF16)
        nc.scalar.dma_start(sqw[:], sqw_d)
        nc.sync.dma_start(w1s[:, I : 4 * I], w1s_d[:, I : 4 * I])
        nc.sync.dma_start(xt[:, HALF:N], xt_d[:, HALF:N])
        nc.sync.dma_start(xt[:, N + HALF : 2 * N], xt_d[:, N + HALF : 2 * N])
        nc.sync.dma_start(w1s[:, 4 * I :], w1s_d[:, 4 * I :])
        for b in range(2, BLOC):
            nc.sync.dma_start(xt[:, b * N : (b + 1) * N], xt_d[:, b * N : (b + 1) * N])
        ba48 = consts.tile([48, 1], F32)
        nc.scalar.dma_start(ba48[:], ba48_d)
        loct = consts.tile([L, BLOC], F32)
        nc.scalar.dma_start(loct[:], loct_d)
        wpl = consts.tile([L, 1], F32)
        nc.scalar.dma_start(wpl[:], wpl_d)
        sel = consts.tile([BLOC, NGRP * 48], F32)
        nc.scalar.dma_start(sel[:], sel_d)
        bp = consts.tile([BLOC, 1], F32)
        nc.scalar.dma_start(bp[:], bp_d)

        ident = consts.tile([48, 48], F32)
        masks.make_identity(nc, ident[:])
        y_wide = consts.tile([48, NGRP], F32)
        ldbw = consts.tile([48, NGRP], F32)

        # ---- HAM warm-up ----
        # The PE clock sits at 1.2 GHz until ~3.4us of sustained activity.
        # While the first input DMAs are in flight, run throwaway matmuls on
        # a zeroed tile so the real matmuls start at 2.4 GHz.
        warm_in = consts.tile([I, HALF], BF16)
        nc.gpsimd.memset(warm_in[:], 0.0)
        warm_ps = zpool.tile([I, HALF], F32, tag="z", bufs=6)
        for _ in range(10):
            nc.tensor.matmul(
                warm_ps[:], warm_in[:, :I], warm_in[:], start=True, stop=True
            )

        # ---- per-batch-pair epilogue, deferred and drip-fed into the next
        # pair's class loop so its serial ACT/DVE chain rides the slack ----
        def make_epilogue(g, sq_ps):
            # NB: tensor_scalar/TTR cannot encode wide PSUM reads at nonzero
            # partition bases, but tensor_tensor / copies / activation can
            # (probed on HW); all slice bases here are 32-aligned.
            s_relu = epool.tile([48, N], F32, tag="srelu")
            e = epool.tile([48, N], F32, tag="e")
            prod = epool.tile([48, N], BF16, tag="prod")
            dump = epool.tile([48, N], BF16, tag="dump")
            den = epool.tile([48, 1], F32, tag="den")
            num = epool.tile([48, 1], F32, tag="num")
            rden = epool.tile([48, 1], F32, tag="rden")
            u = epool.tile([48, 1], F32, tag="u")
            t = epool.tile([48, 1], F32, tag="t")
            t1 = epool.tile([48, 1], F32, tag="t1")

            def p0():
                nc.scalar.activation(
                    s_relu[:, 0:HALF], sq_ps[0][0:48, :], AF.Relu, bias=ba48[:, 0:1]
                )

            def p1():
                nc.vector.tensor_scalar(
                    s_relu[:, HALF:N],
                    sq_ps[1][0:48, :],
                    ba48[:, 0:1],
                    0.0,
                    op0=OP.add,
                    op1=OP.max,
                )

            def p2():
                nc.scalar.activation(e[:], s_relu[:], AF.Exp, accum_out=den[:])

            def p3():
                nc.vector.tensor_tensor(
                    prod[:, 0:HALF], e[:, 0:HALF], sq_ps[0][64:112, :], op=OP.mult
                )

            def p4():
                nc.vector.tensor_tensor(
                    prod[:, HALF:N], e[:, HALF:N], sq_ps[1][64:112, :], op=OP.mult
                )

            def p5():
                nc.vector.tensor_scalar(
                    dump[:],
                    prod[:],
                    1.0,
                    0.0,
                    op0=OP.mult,
                    op1=OP.add,
                    accum_out=num[:],
                )

            def p6():
                nc.vector.reciprocal(rden[:], den[:])
                nc.vector.tensor_scalar(
                    u[:],
                    num[:],
                    rden[:, 0:1],
                    ldbw[:, g : g + 1],
                    op0=OP.mult,
                    op1=OP.add,
                )

            def p7():
                # sigmoid(u) = 1 / (1 + exp(-u))  (stay in the exp table set)
                nc.scalar.activation(t[:], u[:], AF.Exp, scale=-1.0)
                nc.vector.tensor_scalar(t1[:], t[:], 1.0, None, op0=OP.add)

            def p8():
                nc.vector.reciprocal(y_wide[:, g : g + 1], t1[:])

            return [p0, p1, p2, p3, p4, p5, p6, p7, p8]

        # ---- main pipeline ----
        # Unit u of a batch pair = (class c = u>>1, parity par = u&1), batch
        # b = 2*g + par.  The s/q matmuls trail the z matmuls + relu by DEPTH
        # units so the PE never waits on a relu; matmuls stay back-to-back.
        kk = 0  # ACT/DVE relu round-robin counter (ACT gets 17 of 32 halves)
        DEPTH = 8
        pending = []
        for g in range(NGRP):
            sq_ps = [
                sqpool.tile([M2, HALF], F32, name=f"sq{hf}", tag=f"sq{hf}")
                for hf in range(2)
            ]
            hts = {}
            for uu in range(NU + DEPTH):
                if uu < NU:
                    c, par = uu >> 1, uu & 1
                    b = 2 * g + par
                    ht = htpool.tile([I, N], BF16)
                    hts[uu] = ht
                    for hf in range(2):
                        lo = hf * HALF
                        z = zpool.tile([I, HALF], F32, tag="z", bufs=6)
                        nc.tensor.matmul(
                            z[:],
                            w1s[:, c * I : (c + 1) * I],
                            xt[:, b * N + lo : b * N + lo + HALF],
                            start=True,
                            stop=True,
                        )
                        hslice = ht[:, lo : lo + HALF]
                        if (kk * 17) % 32 < 17:
                            nc.scalar.activation(
                                hslice, z[:], AF.Relu, bias=b1t[:, c : c + 1]
                            )
                        else:
                            nc.vector.tensor_scalar(
                                hslice,
                                z[:],
                                b1t[:, c : c + 1],
                                0.0,
                                op0=OP.add,
                                op1=OP.max,
                            )
                        kk += 1
                if g == 0 and uu == 16:
                    # ldbw[m, gg] = Wp_L @ loc[2gg + (m>=32)] + bp.  Emitted
                    # mid-loop so its serial matmul chain overlaps the unit
                    # stream instead of stalling the PE at the group boundary.
                    ldc_ps = zpool.tile([BLOC, 1], F32, tag="z", bufs=6)
                    nc.tensor.matmul(ldc_ps[:], loct[:], wpl[:], start=True, stop=True)
                    ldc = consts.tile([BLOC, 1], F32)
                    nc.scalar.activation(
                        ldc[:], ldc_ps[:], AF.Identity, bias=bp[:, 0:1]
                    )
                    ldbw_ps = zpool.tile([48, NGRP], F32, tag="z", bufs=6)
                    for gg in range(NGRP):
                        nc.tensor.matmul(
                            ldbw_ps[:, gg : gg + 1],
                            sel[:, gg * 48 : (gg + 1) * 48],
                            ldc[:],
                            start=True,
                            stop=True,
                        )
                    nc.vector.tensor_copy(ldbw[:], ldbw_ps[:])
                # drip-feed the previous pair's epilogue into this loop
                # (two pieces per step, emitted before this pair's s/q MMs so
                # the sq_ps PSUM tiles are released before reuse at uu=DEPTH)
                if pending and uu >= 3:
                    pending.pop(0)()
                    if pending:
                        pending.pop(0)()
                if uu >= DEPTH:
                    v = uu - DEPTH
                    ht = hts.pop(v)
                    for hf in range(2):
                        lo = hf * HALF
                        nc.tensor.matmul(
                            sq_ps[hf][:],
                            sqw[:, v * M2 : (v + 1) * M2],
                            ht[:, lo : lo + HALF],
                            start=(v == 0),
                            stop=(v == NU - 1),
                        )
            while pending:
                pending.pop(0)()
            if False:
                # ldbw[m, g] = Wp_L @ loc[2g + (m>=32)] + bp via two small
                # matmuls: ld per batch on partitions (loct as stationary),
                # then host-provided selection matrices spread it to the
                # 32-aligned row blocks.  Emitted here (not at kernel start)
                # so the PE stream doesn't stall on the small late DMAs.
                ldc_ps = zpool.tile([BLOC, 1], F32, tag="z", bufs=6)
                nc.tensor.matmul(ldc_ps[:], loct[:], wpl[:], start=True, stop=True)
                ldc = consts.tile([BLOC, 1], F32)
                nc.scalar.activation(
                    ldc[:], ldc_ps[:], AF.Identity, bias=bp[:, 0:1]
                )
                ldbw_ps = zpool.tile([48, NGRP], F32, tag="z", bufs=6)
                for gg in range(NGRP):
                    nc.tensor.matmul(
                        ldbw_ps[:, gg : gg + 1],
                        sel[:, gg * 48 : (gg + 1) * 48],
                        ldc[:],
                        start=True,
                        stop=True,
                    )
                nc.vector.tensor_copy(ldbw[:], ldbw_ps[:])
            pending = make_epilogue(g, sq_ps)
        for p in pending:
            p()

        # ---- transpose [48, NGRP] -> [NGRP, 48] and store the two valid
        # 16-column blocks per row ----
        yt_ps = zpool.tile([NGRP, 48], F32, tag="z", bufs=6)
        nc.tensor.transpose(yt_ps[:], y_wide[:], ident[:])
        y_out = consts.tile([NGRP, 48], F32)
        nc.scalar.copy(y_out[:], yt_ps[:])
        y_src = y_out.rearrange("p (k s) -> p k s", s=16)[:, 0:3:2, :]
        nc.sync.dma_start(y_d.rearrange("(g r) c -> g r c", r=2), y_src)


_NC_CACHE = {}


def _get_nc():
    if "nc" not in _NC_CACHE:
        nc = bacc.Bacc(
            "TRN2",
            target_bir_lowering=False,
            debug=False,
            enable_asserts=False,
            num_devices=NCORES,
        )
        with tile.TileContext(nc) as tc:
            _build_kernel(tc)
        nc.compile()
        _NC_CACHE["nc"] = nc
    return _NC_CACHE["nc"]


def _prep_inputs(bags, loc, W1, b1, Wa, ba, Wp, bp):
    """Host-side layout prep (transposes / casts / block packing)."""
    bags = np.asarray(bags, np.float32)
    loc = np.asarray(loc, np.float32).reshape(B, L)
    W1 = np.asarray(W1, np.float32)
    b1 = np.asarray(b1, np.float32)
    Wa = np.asarray(Wa, np.float32)
    ba = np.asarray(ba, np.float32)
    Wp = np.asarray(Wp, np.float32)
    bp = np.asarray(bp, np.float32)

    bf = ml_dtypes.bfloat16
    w1s = np.ascontiguousarray(W1.transpose(1, 0, 2).reshape(I, C * I)).astype(bf)
    b1t = np.ascontiguousarray(b1.T)  # [I, C] f32
    # paired block-column stationary: unit (c, par) puts Wa[c] at column
    # 32*par + c and Wp[:I] at column 64 + 32*par + c
    sqw = np.zeros((I, NU, M2), np.float32)
    for c in range(C):
        for par in range(2):
            u = 2 * c + par
            sqw[:, u, 32 * par + c] = Wa[c]
            sqw[:, u, 64 + 32 * par + c] = Wp[:I]
    sqw = sqw.reshape(I, NU * M2).astype(bf)
    ba48 = np.zeros((48, 1), np.float32)
    ba48[0:16, 0] = ba
    ba48[32:48, 0] = ba
    wpl = np.ascontiguousarray(Wp[I:].reshape(L, 1))
    # selection matrices: sel[:, g*48+m] = 1 iff batch 2g + (m>=32) matches
    sel = np.zeros((BLOC, NGRP, 48), np.float32)
    for g in range(NGRP):
        sel[2 * g, g, 0:16] = 1.0
        sel[2 * g + 1, g, 32:48] = 1.0
    sel = sel.reshape(BLOC, NGRP * 48)
    bp2 = np.full((BLOC, 1), float(bp), np.float32)

    in_maps = []
    for k in range(NCORES):
        sl = slice(k * BLOC, (k + 1) * BLOC)
        xt = np.ascontiguousarray(
            bags[sl].transpose(2, 0, 1).reshape(I, BLOC * N)
        ).astype(bf)
        loct = np.ascontiguousarray(loc[sl].T)  # [L, BLOC]
        in_maps.append(
            dict(
                xt=xt,
                w1s=w1s,
                b1t=b1t,
                sqw=sqw,
                ba48=ba48,
                loct=loct,
                wpl=wpl,
                sel=sel,
                bp=bp2,
            )
        )
    return in_maps


def run(bags, loc, W1, b1, Wa, ba, Wp, bp, **run_kwargs):
    """Run on 8 cores; returns (y [B, C] fp32, BassKernelResults)."""
    nc = _get_nc()
    in_maps = _prep_inputs(bags, loc, W1, b1, Wa, ba, Wp, bp)
    res = bass_utils.run_bass_kernel_spmd(
        nc, in_maps, core_ids=list(range(NCORES)), **run_kwargs
    )
    y = np.concatenate([res.results[k]["y"] for k in range(NCORES)], axis=0)
    return y.astype(np.float32), res


def kernel(bags, loc, W1, b1, Wa, ba, Wp, bp):
    y, _ = run(bags, loc, W1, b1, Wa, ba, Wp, bp)
    return y


# revision 40
# speedup vs baseline: 1.0225x; 1.0225x over previous
"""Trainium2 Bass kernel for nn_Model_17325898072228 (attention-MIL pooling).

Math (per batch b, class c):
    h      = relu(bags[b] @ W1[c] + b1[c])            # [N, I]
    s      = relu(h @ Wa[c] + ba[c])                  # [N]
    w      = softmax(s)                               # [N]
    pooled = w @ h                                    # [I]
    y      = sigmoid(Wp[:I]@pooled + Wp[I:]@loc[b] + bp)

Key algebraic reduction used on-device: the pooled vector is never needed,
only its dot with Wp:
    Wp_I @ pooled = (sum_n e_n * q_n) / (sum_n e_n)
with e_n = exp(relu(s_n)) (softmax shift is unnecessary: scores are O(1))
and q_n = Wp_I @ h_n.  Both s and q are j-contractions of h, so one matmul
per (batch, class) with a [128, 112] block-column stationary accumulates,
for a PAIR of batches, s rows at PSUM partitions {0-15, 32-47} and q rows
at {64-79, 96-111} for all 16 classes.  All partition bases are 32-aligned
(a hardware requirement for wide PSUM reads), junk rows in between are
processed harmlessly in parallel, and the whole softmax/dot epilogue runs
once per batch-pair instead of once per batch.  No transpose of h is ever
required.

Sharding: data-parallel over batch B=32 across 8 cores (4 batches each).
All weights are replicated; inputs are laid out host-side (transposes,
bf16 casts, block-column stationary assembly) so on-device DMA is
contiguous.
"""

import sys

sys.path.insert(0, "/opt/trn_rl_repo")

from contextlib import ExitStack

import ml_dtypes
import numpy as np

import concourse.bass as bass  # noqa: F401  (registers engines)
import concourse.tile as tile
from concourse import bacc, bass_utils, masks, mybir


B, N, I, L, C = 32, 1024, 128, 32, 16
NCORES = 8
BLOC = B // NCORES  # batches per core
NGRP = BLOC // 2  # batch pairs per core
HALF = 512  # matmul moving-operand max free dim (= one fp32 psum bank)
M2 = 112  # stationary width for the paired s/q matmul
NU = 2 * C  # (class, parity) units per batch-pair

F32 = mybir.dt.float32
BF16 = mybir.dt.bfloat16
AF = mybir.ActivationFunctionType
OP = mybir.AluOpType


def _build_kernel(tc):
    nc = tc.nc
    xt_d = nc.dram_tensor("xt", [I, BLOC * N], BF16, kind="ExternalInput").ap()
    w1s_d = nc.dram_tensor("w1s", [I, C * I], BF16, kind="ExternalInput").ap()
    b1t_d = nc.dram_tensor("b1t", [I, C], F32, kind="ExternalInput").ap()
    sqw_d = nc.dram_tensor("sqw", [I, NU * M2], BF16, kind="ExternalInput").ap()
    ba48_d = nc.dram_tensor("ba48", [48, 1], F32, kind="ExternalInput").ap()
    loct_d = nc.dram_tensor("loct", [L, BLOC], F32, kind="ExternalInput").ap()
    wpl_d = nc.dram_tensor("wpl", [L, 1], F32, kind="ExternalInput").ap()
    sel_d = nc.dram_tensor("sel", [BLOC, NGRP * 48], F32, kind="ExternalInput").ap()
    bp_d = nc.dram_tensor("bp", [BLOC, 1], F32, kind="ExternalInput").ap()
    y_d = nc.dram_tensor("y", [BLOC, C], F32, kind="ExternalOutput").ap()

    with ExitStack() as ctx:
        consts = ctx.enter_context(tc.tile_pool(name="consts", bufs=1))
        zpool = ctx.enter_context(tc.tile_pool(name="z", bufs=6, space="PSUM"))
        sqpool = ctx.enter_context(tc.tile_pool(name="sq", bufs=1, space="PSUM"))
        htpool = ctx.enter_context(tc.tile_pool(name="ht", bufs=12))
        epool = ctx.enter_context(tc.tile_pool(name="ep", bufs=3))

        # ---- persistent loads, ordered/split so the first matmuls can
        # start early; spread across both HWDGE rings (sync + scalar) ----
        w1s = consts.tile([I, C * I], BF16)
        nc.sync.dma_start(w1s[:, :I], w1s_d[:, :I])
        xt = consts.tile([I, BLOC * N], BF16)
        nc.scalar.dma_start(xt[:, :HALF], xt_d[:, :HALF])
        nc.sync.dma_start(xt[:, N : N + HALF], xt_d[:, N : N + HALF])
        b1t = consts.tile([I, C], F32)
        nc.scalar.dma_start(b1t[:], b1t_d)
        sqw = consts.tile([I, NU * M2], BF16)
        nc.scalar.dma_start(sqw[:], sqw_d)
        nc.sync.dma_start(w1s[:, I : 4 * I], w1s_d[:, I : 4 * I])
        nc.sync.dma_start(xt[:, HALF:N], xt_d[:, HALF:N])
        nc.sync.dma_start(xt[:, N + HALF : 2 * N], xt_d[:, N + HALF : 2 * N])
        nc.sync.dma_start(w1s[:, 4 * I :], w1s_d[:, 4 * I :])
        for b in range(2, BLOC):
            nc.sync.dma_start(xt[:, b * N : (b + 1) * N], xt_d[:, b * N : (b + 1) * N])
        ba48 = consts.tile([48, 1], F32)
        nc.scalar.dma_start(ba48[:], ba48_d)
        loct = consts.tile([L, BLOC], F32)
        nc.scalar.dma_start(loct[:], loct_d)
        wpl = consts.tile([L, 1], F32)
        nc.scalar.dma_start(wpl[:], wpl_d)
        sel = consts.tile([BLOC, NGRP * 48], F32)
        nc.scalar.dma_start(sel[:], sel_d)
        bp = consts.tile([BLOC, 1], F32)
        nc.scalar.dma_start(bp[:], bp_d)

        ident = consts.tile([48, 48], F32)
        masks.make_identity(nc, ident[:])
        y_wide = consts.tile([48, NGRP], F32)
        ldbw = consts.tile([48, NGRP], F32)

        # ---- HAM warm-up ----
        # The PE clock sits at 1.2 GHz until ~3.4us of sustained activity.
        # While the first input DMAs are in flight, run throwaway matmuls on
        # a zeroed tile so the real matmuls start at 2.4 GHz.
        warm_in = consts.tile([I, HALF], BF16)
        nc.gpsimd.memset(warm_in[:], 0.0)
        warm_ps = zpool.tile([I, HALF], F32, tag="z", bufs=6)
        for _ in range(10):
            nc.tensor.matmul(
                warm_ps[:], warm_in[:, :I], warm_in[:], start=True, stop=True
            )

        # ---- per-batch-pair epilogue, deferred and drip-fed into the next
        # pair's class loop so its serial ACT/DVE chain rides the slack ----
        def make_epilogue(g, sq_ps):
            # NB: tensor_scalar/TTR cannot encode wide PSUM reads at nonzero
            # partition bases, but tensor_tensor / copies / activation can
            # (probed on HW); all slice bases here are 32-aligned.
            s_relu = epool.tile([48, N], F32, tag="srelu")
            e = epool.tile([48, N], F32, tag="e")
            prod = epool.tile([48, N], BF16, tag="prod")
            dump = epool.tile([48, N], BF16, tag="dump")
            den = epool.tile([48, 1], F32, tag="den")
            num = epool.tile([48, 1], F32, tag="num")
            rden = epool.tile([48, 1], F32, tag="rden")
            u = epool.tile([48, 1], F32, tag="u")
            t = epool.tile([48, 1], F32, tag="t")
            t1 = epool.tile([48, 1], F32, tag="t1")

            def p0():
                nc.scalar.activation(
                    s_relu[:, 0:HALF], sq_ps[0][0:48, :], AF.Relu, bias=ba48[:, 0:1]
                )

            def p1():
                nc.vector.tensor_scalar(
                    s_relu[:, HALF:N],
                    sq_ps[1][0:48, :],
                    ba48[:, 0:1],
                    0.0,
                    op0=OP.add,
                    op1=OP.max,
                )

            def p2():
                nc.scalar.activation(e[:], s_relu[:], AF.Exp, accum_out=den[:])

            def p3():
                nc.vector.tensor_tensor(
                    prod[:, 0:HALF], e[:, 0:HALF], sq_ps[0][64:112, :], op=OP.mult
                )

            def p4():
                nc.vector.tensor_tensor(
                    prod[:, HALF:N], e[:, HALF:N], sq_ps[1][64:112, :], op=OP.mult
                )

            def p5():
                nc.vector.tensor_scalar(
                    dump[:],
                    prod[:],
                    1.0,
                    0.0,
                    op0=OP.mult,
                    op1=OP.add,
                    accum_out=num[:],
                )

            def p6():
                nc.vector.reciprocal(rden[:], den[:])
                nc.vector.tensor_scalar(
                    u[:],
                    num[:],
                    rden[:, 0:1],
                    ldbw[:, g : g + 1],
                    op0=OP.mult,
                    op1=OP.add,
                )

            def p7():
                # sigmoid(u) = 1 / (1 + exp(-u))  (stay in the exp table set)
                nc.scalar.activation(t[:], u[:], AF.Exp, scale=-1.0)
                nc.vector.tensor_scalar(t1[:], t[:], 1.0, None, op0=OP.add)

            def p8():
                nc.vector.reciprocal(y_wide[:, g : g + 1], t1[:])

            return [p0, p1, p2, p3, p4, p5, p6, p7, p8]

        # ---- main pipeline ----
        # Unit u of a batch pair = (class c = u>>1, parity par = u&1), batch
        # b = 2*g + par.  The s/q matmuls trail the z matmuls + relu by DEPTH
        # units so the PE never waits on a relu; matmuls stay back-to-back.
        kk = 0  # ACT/DVE relu round-robin counter (ACT gets 17 of 32 halves)
        DEPTH = 8
        pending = []
        for g in range(NGRP):
            sq_ps = [
                sqpool.tile([M2, HALF], F32, name=f"sq{hf}", tag=f"sq{hf}")
                for hf in range(2)
            ]
            hts = {}
            for uu in range(NU + DEPTH):
                if uu < NU:
                    c, par = uu >> 1, uu & 1
                    b = 2 * g + par
                    ht = htpool.tile([I, N], BF16)
                    hts[uu] = ht
                    for hf in range(2):
                        lo = hf * HALF
                        z = zpool.tile([I, HALF], F32, tag="z", bufs=6)
                        nc.tensor.matmul(
                            z[:],
                            w1s[:, c * I : (c + 1) * I],
                            xt[:, b * N + lo : b * N + lo + HALF],
                            start=True,
                            stop=True,
                        )
                        hslice = ht[:, lo : lo + HALF]
                        if (kk * 17) % 32 < 17:
                            nc.scalar.activation(
                                hslice, z[:], AF.Relu, bias=b1t[:, c : c + 1]
                            )
                        else:
                            nc.vector.tensor_scalar(
                                hslice,
                                z[:],
                                b1t[:, c : c + 1],
                                0.0,
                                op0=OP.add,
                                op1=OP.max,
                            )
                        kk += 1
                # drip-feed the previous pair's epilogue into this loop
                # (two pieces per step, emitted before this pair's s/q MMs so
                # the sq_ps PSUM tiles are released before reuse at uu=DEPTH)
                if pending and uu >= 3:
                    pending.pop(0)()
                    if pending:
                        pending.pop(0)()
                if uu >= DEPTH:
                    v = uu - DEPTH
                    ht = hts.pop(v)
                    for hf in range(2):
                        lo = hf * HALF
                        nc.tensor.matmul(
                            sq_ps[hf][:],
                            sqw[:, v * M2 : (v + 1) * M2],
                            ht[:, lo : lo + HALF],
                            start=(v == 0),
                            stop=(v == NU - 1),
                        )
            while pending:
                pending.pop(0)()
            if g == 0:
                # ldbw[m, g] = Wp_L @ loc[2g + (m>=32)] + bp via two small
                # matmuls: ld per batch on partitions (loct as stationary),
                # then host-provided selection matrices spread it to the
                # 32-aligned row blocks.  Emitted here (not at kernel start)
                # so the PE stream doesn't stall on the small late DMAs.
                ldc_ps = zpool.tile([BLOC, 1], F32, tag="z", bufs=6)
                nc.tensor.matmul(ldc_ps[:], loct[:], wpl[:], start=True, stop=True)
                ldc = consts.tile([BLOC, 1], F32)
                nc.scalar.activation(
                    ldc[:], ldc_ps[:], AF.Identity, bias=bp[:, 0:1]
                )
                ldbw_ps = zpool.tile([48, NGRP], F32, tag="z", bufs=6)
                for gg in range(NGRP):
                    nc.tensor.matmul(
                        ldbw_ps[:, gg : gg + 1],
                        sel[:, gg * 48 : (gg + 1) * 48],
                        ldc[:],
                        start=True,
                        stop=True,
                    )
                nc.vector.tensor_copy(ldbw[:], ldbw_ps[:])
            pending = make_epilogue(g, sq_ps)
        for p in pending:
            p()

        # ---- transpose [48, NGRP] -> [NGRP, 48] and store the two valid
        # 16-column blocks per row ----
        yt_ps = zpool.tile([NGRP, 48], F32, tag="z", bufs=6)
        nc.tensor.transpose(yt_ps[:], y_wide[:], ident[:])
        y_out = consts.tile([NGRP, 48], F32)
        nc.scalar.copy(y_out[:], yt_ps[:])
        y_src = y_out.rearrange("p (k s) -> p k s", s=16)[:, 0:3:2, :]
        nc.sync.dma_start(y_d.rearrange("(g r) c -> g r c", r=2), y_src)


_NC_CACHE = {}


def _get_nc():
    if "nc" not in _NC_CACHE:
        nc = bacc.Bacc(
            "TRN2",
            target_bir_lowering=False,
            debug=False,
            enable_asserts=False,
            num_devices=NCORES,
        )
        with tile.TileContext(nc) as tc:
            _build_kernel(tc)
        nc.compile()
        _NC_CACHE["nc"] = nc
    return _NC_CACHE["nc"]


def _prep_inputs(bags, loc, W1, b1, Wa, ba, Wp, bp):
    """Host-side layout prep (transposes / casts / block packing)."""
    bags = np.asarray(bags, np.float32)
    loc = np.asarray(loc, np.float32).reshape(B, L)
    W1 = np.asarray(W1, np.float32)
    b1 = np.asarray(b1, np.float32)
    Wa = np.asarray(Wa, np.float32)
    ba = np.asarray(ba, np.float32)
    Wp = np.asarray(Wp, np.float32)
    bp = np.asarray(bp, np.float32)

    bf = ml_dtypes.bfloat16
    w1s = np.ascontiguousarray(W1.transpose(1, 0, 2).reshape(I, C * I)).astype(bf)
    b1t = np.ascontiguousarray(b1.T)  # [I, C] f32
    # paired block-column stationary: unit (c, par) puts Wa[c] at column
    # 32*par + c and Wp[:I] at column 64 + 32*par + c
    sqw = np.zeros((I, NU, M2), np.float32)
    for c in range(C):
        for par in range(2):
            u = 2 * c + par
            sqw[:, u, 32 * par + c] = Wa[c]
            sqw[:, u, 64 + 32 * par + c] = Wp[:I]
    sqw = sqw.reshape(I, NU * M2).astype(bf)
    ba48 = np.zeros((48, 1), np.float32)
    ba48[0:16, 0] = ba
    ba48[32:48, 0] = ba
    wpl = np.ascontiguousarray(Wp[I:].reshape(L, 1))
    # selection matrices: sel[:, g*48+m] = 1 iff batch 2g + (m>=32) matches
    sel = np.zeros((BLOC, NGRP, 48), np.float32)
    for g in range(NGRP):
        sel[2 * g, g, 0:16] = 1.0
        sel[2 * g + 1, g, 32:48] = 1.0
    sel = sel.reshape(BLOC, NGRP * 48)
    bp2 = np.full((BLOC, 1), float(bp), np.float32)

    in_maps = []
    for k in range(NCORES):
        sl = slice(k * BLOC, (k + 1) * BLOC)
        xt = np.ascontiguousarray(
            bags[sl].transpose(2, 0, 1).reshape(I, BLOC * N)
        ).astype(bf)
        loct = np.ascontiguousarray(loc[sl].T)  # [L, BLOC]
        in_maps.append(
            dict(
                xt=xt,
                w1s=w1s,
                b1t=b1t,
                sqw=sqw,
                ba48=ba48,
                loct=loct,
                wpl=wpl,
                sel=sel,
                bp=bp2,
            )
        )
    return in_maps


def run(bags, loc, W1, b1, Wa, ba, Wp, bp, **run_kwargs):
    """Run on 8 cores; returns (y [B, C] fp32, BassKernelResults)."""
    nc = _get_nc()
    in_maps = _prep_inputs(bags, loc, W1, b1, Wa, ba, Wp, bp)
    res = bass_utils.run_bass_kernel_spmd(
        nc, in_maps, core_ids=list(range(NCORES)), **run_kwargs
    )
    y = np.concatenate([res.results[k]["y"] for k in range(NCORES)], axis=0)
    return y.astype(np.float32), res


def kernel(bags, loc, W1, b1, Wa, ba, Wp, bp):
    y, _ = run(bags, loc, W1, b1, Wa, ba, Wp, bp)
    return y
